# revision 7
# baseline (speedup 1.0000x reference)
"""GCN classifier Trainium2 kernel (8-core SPMD, Bass/Tile).

Model (reference):
    h1 = relu(gcnconv(x, W1, b1));  h2 = gcnconv(h1, W2, b2);  out = mean-pool(h2, batch)

Distribution strategy (no cross-core communication):
  * Nodes sharded contiguously across 8 cores (6250 each); x replicated (fp16 copy).
  * Layer 1 aggregation per dst shard: per-tile (128-edge) indirect-DMA gathers of
    x rows; symmetric-norm weight w_e = dinv[src]*dinv[dst] folded into the
    selection matrix S_T[e,d] = w_e * (dst_e == d), built in ONE fused DVE
    tensor_scalar op; scatter = fp16 matmul accumulating into f32 PSUM.
  * h1^T and z2 = h1 @ W2 stay on-chip (PSUM/SBUF), never round-trip HBM.
  * Layer 2 + mean-pool collapse algebraically:
        pool_sums[g,f] = sum_e w_e * z2[src_e, f] * [batch[dst_e] == g]
                       = sum_n C[g,n] * z2[n,f]
    with C built host-side from indices/weights only -> dense matmuls, zero
    communication. Host sums 8 partials, divides by counts, adds b2.

Numerics: fp16 operand quantization (~5e-4) with exact f32 PSUM accumulation.
(True f32 matmuls on TRN2 PE are bf16-class 2-pass approximations anyway.)
"""

import math
import numpy as np

N_NODES = 50000
N_EDGES = 600000
N_GRAPHS = 64
IN_DIM = 128
HID_DIM = 128
OUT_DIM = 64
N_CORES = 8
P = 128


# ---------------------------------------------------------------- host prep
def _host_prep(x, edge_index, batch):
    n = x.shape[0]
    shard = n // N_CORES                    # 6250
    n_blocks = math.ceil(shard / P)         # 49
    shard_pad = n_blocks * P                # 6272

    src = np.asarray(edge_index[0], dtype=np.int64)
    dst = np.asarray(edge_index[1], dtype=np.int64)
    batch = np.asarray(batch, dtype=np.int64)

    deg = np.bincount(dst, minlength=n).astype(np.float32) + np.float32(1.0)
    dinv = (np.float32(1.0) / np.sqrt(deg)).astype(np.float32)

    loops = np.arange(n, dtype=np.int64)
    SRC = np.concatenate([src, loops])
    DST = np.concatenate([dst, loops])
    W = (dinv[SRC] * dinv[DST]).astype(np.float32)
    E = SRC.shape[0]

    # ---- layer-1 bucketing by (core, block) of DST
    core_of = DST // shard
    blk_of = (DST % shard) // P
    dstl = (DST % shard) % P

    bucket = core_of * n_blocks + blk_of
    order = np.argsort(bucket, kind="stable")
    bucket_s = bucket[order]
    counts = np.bincount(bucket_s, minlength=N_CORES * n_blocks)
    T = int(math.ceil(counts.max() / P))    # uniform tiles per block (SPMD)

    cum = np.zeros(N_CORES * n_blocks + 1, dtype=np.int64)
    np.cumsum(counts, out=cum[1:])
    pos = np.arange(E) - cum[bucket_s]

    ncols = n_blocks * T
    src_cols = np.zeros((N_CORES, P, ncols), dtype=np.int32)
    w_cols = np.zeros((N_CORES, P, ncols), dtype=np.float32)
    dstl_cols = np.zeros((N_CORES, P, ncols), dtype=np.float32)

    e_core = core_of[order]
    col = blk_of[order] * T + pos // P
    row = pos % P
    src_cols[e_core, row, col] = SRC[order].astype(np.int32)
    w_cols[e_core, row, col] = W[order]
    dstl_cols[e_core, row, col] = dstl[order].astype(np.float32)

    # ---- layer-2 dense matrix C[g, n] = sum_{e: src=n} w_e * [batch[dst_e]=g]
    g_of = batch[DST]
    idx = ((SRC // shard) * N_GRAPHS + g_of) * shard + (SRC % shard)
    C = np.bincount(idx, weights=W.astype(np.float64),
                    minlength=N_CORES * N_GRAPHS * shard)
    C = C.reshape(N_CORES, N_GRAPHS, shard).astype(np.float32)

    Cp = np.zeros((N_CORES, N_GRAPHS, shard_pad), dtype=np.float32)
    Cp[:, :, :shard] = C
    CT_cols = Cp.reshape(N_CORES, N_GRAPHS, n_blocks, P).transpose(0, 3, 2, 1)
    CT_cols = np.ascontiguousarray(
        CT_cols.reshape(N_CORES, P, n_blocks * N_GRAPHS)).astype(np.float16)

    graph_counts = np.bincount(batch, minlength=N_GRAPHS).astype(np.float32)

    return dict(T=T, n_blocks=n_blocks, shard=shard,
                src_cols=src_cols, w_cols=w_cols, dstl_cols=dstl_cols,
                CT_cols=CT_cols, graph_counts=graph_counts)


# ---------------------------------------------------------------- bass program
_PROGRAM_CACHE = {}


def _build_program(T, n_blocks, n_nodes):
    import concourse.bacc as bacc
    import concourse.tile as tile
    from concourse import mybir
    from concourse.bass import IndirectOffsetOnAxis
    from concourse.masks import make_identity

    f32, i32 = mybir.dt.float32, mybir.dt.int32
    f16 = mybir.dt.float16
    AF = mybir.ActivationFunctionType

    ncols = n_blocks * T

    nc = bacc.Bacc("TRN2", target_bir_lowering=False, debug=False,
                   num_devices=N_CORES)
    x16_d = nc.dram_tensor("x16", [n_nodes, IN_DIM], f16, kind="ExternalInput")
    w1_d = nc.dram_tensor("w1", [IN_DIM, HID_DIM], f16, kind="ExternalInput")
    w2_d = nc.dram_tensor("w2", [HID_DIM, OUT_DIM], f16, kind="ExternalInput")
    b1_d = nc.dram_tensor("b1", [HID_DIM, 1], f32, kind="ExternalInput")
    srcc_d = nc.dram_tensor("src_cols", [P, ncols], i32, kind="ExternalInput")
    wc_d = nc.dram_tensor("w_cols", [P, ncols], f32, kind="ExternalInput")
    dstc_d = nc.dram_tensor("dstl_cols", [P, ncols], f32, kind="ExternalInput")
    ctc_d = nc.dram_tensor("ct_cols", [P, n_blocks * N_GRAPHS], f16,
                           kind="ExternalInput")
    pool_d = nc.dram_tensor("pool_out", [N_GRAPHS, OUT_DIM], f32,
                            kind="ExternalOutput")

    with tile.TileContext(nc) as tc:
        with (
            tc.tile_pool(name="const", bufs=1) as cp,
            tc.tile_pool(name="work", bufs=4) as wp,
            tc.tile_pool(name="gat", bufs=8) as gp,
            tc.tile_pool(name="ps_out1", bufs=2, space="PSUM") as ps1,
            tc.tile_pool(name="ps_misc", bufs=1, space="PSUM") as ps2,
            tc.tile_pool(name="ps_pool", bufs=1, space="PSUM") as psp,
        ):
            iota_i = cp.tile([P, P], i32)
            nc.gpsimd.iota(iota_i[:], pattern=[[1, P]], base=0,
                           channel_multiplier=0)
            iota_f = cp.tile([P, P], f32)
            nc.vector.tensor_copy(out=iota_f[:], in_=iota_i[:])
            ident = cp.tile([P, P], f32)
            make_identity(nc, ident[:])
            w1_t = cp.tile([IN_DIM, HID_DIM], f16)
            nc.sync.dma_start(out=w1_t[:], in_=w1_d[:])
            w2_t = cp.tile([HID_DIM, OUT_DIM], f16)
            nc.sync.dma_start(out=w2_t[:], in_=w2_d[:])
            b1_t = cp.tile([HID_DIM, 1], f32)
            nc.sync.dma_start(out=b1_t[:], in_=b1_d[:])
            srcc = cp.tile([P, ncols], i32)
            nc.sync.dma_start(out=srcc[:], in_=srcc_d[:])
            wc = cp.tile([P, ncols], f32)
            nc.sync.dma_start(out=wc[:], in_=wc_d[:])
            dstc = cp.tile([P, ncols], f32)
            nc.sync.dma_start(out=dstc[:], in_=dstc_d[:])
            ctc = cp.tile([P, n_blocks * N_GRAPHS], f16)
            nc.sync.dma_start(out=ctc[:], in_=ctc_d[:])

            pool_ps = psp.tile([N_GRAPHS, OUT_DIM], f32, space="PSUM")

            for b in range(n_blocks):
                out1 = ps1.tile([P, IN_DIM], f32, space="PSUM", tag="out1")
                for j in range(T):
                    c = b * T + j
                    gat = gp.tile([P, IN_DIM], f16, tag="gat")
                    nc.gpsimd.indirect_dma_start(
                        out=gat[:], out_offset=None, in_=x16_d[:],
                        in_offset=IndirectOffsetOnAxis(ap=srcc[:, c:c + 1],
                                                       axis=0))
                    stw = wp.tile([P, P], f16, tag="stw")
                    nc.vector.tensor_scalar(
                        out=stw[:], in0=iota_f[:],
                        scalar1=dstc[:, c:c + 1], scalar2=wc[:, c:c + 1],
                        op0=mybir.AluOpType.is_equal, op1=mybir.AluOpType.mult)
                    nc.tensor.matmul(out=out1[:], lhsT=stw[:], rhs=gat[:],
                                     start=(j == 0), stop=(j == T - 1))

                # h1T = relu(W1^T @ OUT1^T + b1);  z2 = h1 @ W2;  pool += C^T @ z2
                o1s = wp.tile([P, IN_DIM], f32, tag="o1s")
                nc.scalar.activation(out=o1s[:], in_=out1[:], func=AF.Copy)
                o1t_ps = ps2.tile([IN_DIM, P], f32, space="PSUM", tag="o1t")
                nc.tensor.transpose(out=o1t_ps[:], in_=o1s[:], identity=ident[:])
                o1t = wp.tile([IN_DIM, P], f16, tag="o1t_sb")
                nc.vector.tensor_copy(out=o1t[:], in_=o1t_ps[:])
                h1t_ps = ps2.tile([HID_DIM, P], f32, space="PSUM", tag="h1t")
                nc.tensor.matmul(out=h1t_ps[:], lhsT=w1_t[:], rhs=o1t[:],
                                 start=True, stop=True)
                h1t = wp.tile([HID_DIM, P], f16, tag="h1t_sb")
                nc.scalar.activation(out=h1t[:], in_=h1t_ps[:], func=AF.Relu,
                                     bias=b1_t[:, :1])
                z2_ps = ps2.tile([P, OUT_DIM], f32, space="PSUM", tag="z2")
                nc.tensor.matmul(out=z2_ps[:], lhsT=h1t[:], rhs=w2_t[:],
                                 start=True, stop=True)
                z2s = wp.tile([P, OUT_DIM], f16, tag="z2_sb")
                nc.scalar.activation(out=z2s[:], in_=z2_ps[:], func=AF.Copy)
                nc.tensor.matmul(
                    out=pool_ps[:],
                    lhsT=ctc[:, b * N_GRAPHS:(b + 1) * N_GRAPHS],
                    rhs=z2s[:], start=(b == 0), stop=(b == n_blocks - 1))

            pool_sb = wp.tile([N_GRAPHS, OUT_DIM], f32, tag="pool_sb")
            nc.scalar.activation(out=pool_sb[:], in_=pool_ps[:], func=AF.Copy)
            nc.sync.dma_start(out=pool_d[:], in_=pool_sb[:])

    nc.compile()
    return nc


def _make_in_maps(x, W1, W2, b1, prep):
    x16 = np.ascontiguousarray(x.astype(np.float16))
    b1_col = np.ascontiguousarray(b1.reshape(HID_DIM, 1).astype(np.float32))
    w1_16 = W1.astype(np.float16)
    w2_16 = W2.astype(np.float16)
    in_maps = []
    for c in range(N_CORES):
        in_maps.append({
            "x16": x16,
            "w1": w1_16,
            "w2": w2_16,
            "b1": b1_col,
            "src_cols": np.ascontiguousarray(prep["src_cols"][c]),
            "w_cols": np.ascontiguousarray(prep["w_cols"][c]),
            "dstl_cols": np.ascontiguousarray(prep["dstl_cols"][c]),
            "ct_cols": np.ascontiguousarray(prep["CT_cols"][c]),
        })
    return in_maps


# ---------------------------------------------------------------- entry point
def kernel(x, edge_index, batch, W1, b1, W2, b2):
    from concourse.bass_utils import run_bass_kernel_spmd

    x = np.asarray(x, dtype=np.float32)
    W1 = np.asarray(W1, dtype=np.float32)
    b1 = np.asarray(b1, dtype=np.float32)
    W2 = np.asarray(W2, dtype=np.float32)
    b2 = np.asarray(b2, dtype=np.float32)

    prep = _host_prep(x, edge_index, batch)
    key = (prep["T"], prep["n_blocks"], x.shape[0])
    if key not in _PROGRAM_CACHE:
        _PROGRAM_CACHE[key] = _build_program(*key)
    nc = _PROGRAM_CACHE[key]

    in_maps = _make_in_maps(x, W1, W2, b1, prep)
    res = run_bass_kernel_spmd(nc, in_maps, list(range(N_CORES)))
    globals()["_LAST_RESULT"] = res

    total = np.zeros((N_GRAPHS, OUT_DIM), dtype=np.float64)
    for c in range(N_CORES):
        total += res.results[c]["pool_out"].astype(np.float64)

    counts = np.maximum(prep["graph_counts"], 1.0).astype(np.float32)
    out = (total.astype(np.float32) / counts[:, None]) + b2[None, :]
    return out.astype(np.float32)
